# revision 20
# baseline (speedup 1.0000x reference)
"""Decision Transformer Bass kernel for 8 Trainium2 NeuronCores.

Sharding: data-parallel over batch B=16 -> 2 items per core, params replicated.
All activations live transposed in SBUF: [H partitions, token cols].
Matmuls use float32r views (full PE rate at N>=256, fp32 numerics).
"""

import numpy as np

# model dims (hardcoded per contract)
B, T, SD, AD = 16, 128, 17, 6
H, NH, NB, MAXTS = 512, 8, 6, 4096
D = H // NH          # 64
S = 3 * T            # 384 tokens per item
NC = 8               # cores
BL = B // NC         # 2 items per core
W2 = BL * S          # 768 activation cols per core
KC = H // 128        # 4 k-chunks of hidden dim
FF = 4 * H           # 2048
NEG = -1.0e30
MM_F16 = True   # fp16 matmul operands (fast path); False -> float32r

_BUILT = {}


# ----------------------------------------------------------------------------
# host-side weight packing
# ----------------------------------------------------------------------------

def _lhsT(w):
    """[Kin, M] -> [128, Kin//128, M] chunk layout for lhsT slices."""
    w = np.ascontiguousarray(np.asarray(w, np.float32))
    kin, m = w.shape
    kc = kin // 128
    return np.ascontiguousarray(w.reshape(kc, 128, m).transpose(1, 0, 2))


def _bcol(b, mc=4):
    """[M] bias -> [128, mc] per-partition layout."""
    b = np.asarray(b, np.float32)
    return np.ascontiguousarray(b.reshape(mc, 128).T)


def _pack_inputs(timesteps, states, actions, returns_to_go, params):
    """Returns (shared weight map, list of per-core input maps)."""
    p = params
    shared = {}
    scale_q = 1.0 / np.sqrt(np.float32(D))

    for i in range(NB):
        bp = p['blocks'][i]
        a = bp['attn']
        qkv = np.stack([_lhsT(np.asarray(a['q']['w']) * scale_q),
                        _lhsT(a['k']['w']), _lhsT(a['v']['w'])], axis=1)
        shared[f'qkv_w_{i}'] = np.ascontiguousarray(qkv)          # [128,3,4,512]
        shared[f'o_w_{i}'] = _lhsT(a['o']['w'])                   # [128,4,512]
        qb = np.stack([_bcol(np.asarray(a['q']['b']) * scale_q),
                       _bcol(a['k']['b']), _bcol(a['v']['b']),
                       _bcol(a['o']['b'])], axis=1)
        shared[f'qkvo_b_{i}'] = np.ascontiguousarray(qb)          # [128,4,4]
        shared[f'fc1_w_{i}'] = _lhsT(bp['fc1']['w'])              # [128,4,2048]
        shared[f'fc1_b_{i}'] = _bcol(bp['fc1']['b'], 16)          # [128,16]
        shared[f'fc2_w_{i}'] = _lhsT(bp['fc2']['w'])              # [128,16,512]
        shared[f'fc2_b_{i}'] = _bcol(bp['fc2']['b'])              # [128,4]
        ln = np.stack([np.stack([_bcol(bp['ln1']['scale']), _bcol(bp['ln1']['bias'])], 0),
                       np.stack([_bcol(bp['ln2']['scale']), _bcol(bp['ln2']['bias'])], 0)], 0)
        shared[f'ln_{i}'] = np.ascontiguousarray(ln)              # [128,2,2,4] -> idx [p? see below]

    shared['embln'] = np.ascontiguousarray(
        np.stack([_bcol(p['embed_ln']['scale']), _bcol(p['embed_ln']['bias'])], 0))  # [2,128,4]
    shared['emb_tbl'] = np.ascontiguousarray(np.asarray(p['embed_timestep'], np.float32))
    shared['ws_w'] = np.ascontiguousarray(np.asarray(p['embed_state']['w'], np.float32))   # [17,512]
    shared['wa_w'] = np.ascontiguousarray(np.asarray(p['embed_action']['w'], np.float32))  # [6,512]
    shared['wr_w'] = np.ascontiguousarray(np.asarray(p['embed_rtg']['w'], np.float32))     # [1,512]
    emb_b = np.stack([_bcol(p['embed_rtg']['b']), _bcol(p['embed_state']['b']),
                      _bcol(p['embed_action']['b'])], 0)
    shared['emb_b'] = np.ascontiguousarray(emb_b)                 # [3,128,4] (r,s,a)

    aa = p['align_attn']
    aqkv = np.stack([_lhsT(np.asarray(aa['q']['w']) * scale_q),
                     _lhsT(aa['k']['w']), _lhsT(aa['v']['w'])], axis=1)
    shared['al_qkv_w'] = np.ascontiguousarray(aqkv)
    shared['al_o_w'] = _lhsT(aa['o']['w'])
    aqb = np.stack([_bcol(np.asarray(aa['q']['b']) * scale_q), _bcol(aa['k']['b']),
                    _bcol(aa['v']['b']), _bcol(aa['o']['b'])], axis=1)
    shared['al_qkvo_b'] = np.ascontiguousarray(aqb)               # [128,4,4]
    shared['al_p_w'] = _lhsT(p['align_proj']['w'])                # [128,4,512]
    shared['al_p_b'] = _bcol(p['align_proj']['b'])                # [128,4]

    shared['ps_w'] = _lhsT(p['predict_state']['w'])               # [128,4,17]
    shared['pa_w'] = _lhsT(p['predict_action']['w'])              # [128,4,6]
    shared['pr_w'] = _lhsT(p['predict_rtg']['w'])                 # [128,4,1]
    shared['ps_b'] = np.ascontiguousarray(np.asarray(p['predict_state']['b'], np.float32).reshape(SD, 1))
    shared['pa_b'] = np.ascontiguousarray(np.asarray(p['predict_action']['b'], np.float32).reshape(AD, 1))
    shared['pr_b'] = np.ascontiguousarray(np.asarray(p['predict_rtg']['b'], np.float32).reshape(1, 1))

    # additive causal mask for diagonal 128x128 tiles
    m = np.zeros((128, 128), np.float32)
    m[np.triu_indices(128, 1)] = NEG
    shared['cmask'] = m

    ts = np.asarray(timesteps).astype(np.int32)           # [16,128]
    st = np.asarray(states, np.float32)                   # [16,128,17]
    ac = np.asarray(actions, np.float32)
    rt = np.asarray(returns_to_go, np.float32)

    in_maps = []
    for c in range(NC):
        sl = slice(c * BL, (c + 1) * BL)
        im = dict(shared)
        im['ts_idx'] = np.ascontiguousarray(ts[sl].reshape(BL * T, 1))
        # [feat, item*T + t]
        im['states_t'] = np.ascontiguousarray(st[sl].reshape(BL * T, SD).T)
        im['actions_t'] = np.ascontiguousarray(ac[sl].reshape(BL * T, AD).T)
        im['rtg_t'] = np.ascontiguousarray(rt[sl].reshape(BL * T, 1).T)
        in_maps.append(im)
    if MM_F16:
        f16_names = (['al_qkv_w', 'al_o_w', 'al_p_w', 'ws_w', 'wa_w', 'wr_w',
                      'ps_w', 'pa_w', 'pr_w']
                     + [f'{n}_{i}' for i in range(NB) for n in ('qkv_w', 'o_w', 'fc1_w', 'fc2_w')])
        for nm in f16_names:
            shared[nm] = shared[nm].astype(np.float16)
        for im in in_maps:
            for nm in f16_names:
                im[nm] = shared[nm]
            for nm in ('states_t', 'actions_t', 'rtg_t'):
                im[nm] = im[nm].astype(np.float16)
    return in_maps


# ----------------------------------------------------------------------------
# device program
# ----------------------------------------------------------------------------

def _build(n_blocks=NB, tail=True, debug_h=False):
    import concourse.bass as bass
    import concourse.tile as tile
    from concourse import bacc, mybir
    from concourse.masks import make_identity

    dt = mybir.dt
    F32 = dt.float32
    F32R = dt.float32r
    MMDT = dt.float16 if MM_F16 else F32R
    WDT = dt.float16 if MM_F16 else F32
    AL = mybir.AluOpType
    AF = mybir.ActivationFunctionType

    nc = bacc.Bacc("TRN2", target_bir_lowering=False, debug=False)

    def din(name, shape, dty=F32):
        return nc.dram_tensor(name, list(shape), dty, kind="ExternalInput").ap()

    def dout(name, shape, dty=F32):
        return nc.dram_tensor(name, list(shape), dty, kind="ExternalOutput").ap()

    # --- dram io ---
    ts_idx = din('ts_idx', [BL * T, 1], dt.int32)
    states_t = din('states_t', [SD, BL * T], WDT)
    actions_t = din('actions_t', [AD, BL * T], WDT)
    rtg_t = din('rtg_t', [1, BL * T], WDT)
    emb_tbl = din('emb_tbl', [MAXTS, H])
    ws_w = din('ws_w', [SD, H], WDT); wa_w = din('wa_w', [AD, H], WDT); wr_w = din('wr_w', [1, H], WDT)
    emb_b = din('emb_b', [3, 128, 4])
    embln = din('embln', [2, 128, 4])
    cmask_d = din('cmask', [128, 128])
    blk_w = []
    for i in range(n_blocks):
        blk_w.append(dict(
            qkv=din(f'qkv_w_{i}', [128, 3, KC, H], WDT),
            o=din(f'o_w_{i}', [128, KC, H], WDT),
            qkvo_b=din(f'qkvo_b_{i}', [128, 4, 4]),
            fc1=din(f'fc1_w_{i}', [128, KC, FF], WDT),
            fc1_b=din(f'fc1_b_{i}', [128, 16]),
            fc2=din(f'fc2_w_{i}', [128, 16, H], WDT),
            fc2_b=din(f'fc2_b_{i}', [128, 4]),
            ln=din(f'ln_{i}', [2, 2, 128, 4]),
        ))
    # unused input names still must be declared if provided? we only pass what we declare.
    if tail:
        al_qkv_w = din('al_qkv_w', [128, 3, KC, H], WDT)
        al_o_w = din('al_o_w', [128, KC, H], WDT)
        al_qkvo_b = din('al_qkvo_b', [128, 4, 4])
        al_p_w = din('al_p_w', [128, KC, H], WDT)
        al_p_b = din('al_p_b', [128, 4])
        ps_w = din('ps_w', [128, KC, SD], WDT); ps_b = din('ps_b', [SD, 1])
        pa_w = din('pa_w', [128, KC, AD], WDT); pa_b = din('pa_b', [AD, 1])
        pr_w = din('pr_w', [128, KC, 1], WDT); pr_b = din('pr_b', [1, 1])

    attns = dout('attns', [n_blocks, BL, NH, S, S], dt.float16) if n_blocks else None
    if tail:
        sp_o = dout('sp', [BL, SD, T])
        ap_o = dout('ap_', [BL, AD, T])
        rp_o = dout('rp', [BL, 1, T])
        sproj_o = dout('sproj', [BL, H, T])
        aproj_o = dout('aproj', [BL, H, T])
    if debug_h:
        hdbg = dout('hdbg', [128, KC, W2], MMDT)

    def mm(out, lhsT, rhs, **kw):
        nc.tensor.matmul(out, lhsT, rhs, **kw)

    with tile.TileContext(nc) as tc:
        import contextlib
        ctx = contextlib.ExitStack()
        with ctx:
            ctx.enter_context(nc.allow_low_precision(reason="f32r matmul operands (tf32-like, intended)"))
            p_act = ctx.enter_context(tc.tile_pool(name="act", bufs=2))
            p_qk = ctx.enter_context(tc.tile_pool(name="qk", bufs=3))
            p_vn = ctx.enter_context(tc.tile_pool(name="vn", bufs=1))
            p_w = ctx.enter_context(tc.tile_pool(name="wsb", bufs=3))
            p_wt = ctx.enter_context(tc.tile_pool(name="wtsb", bufs=3))
            p_wgt = ctx.enter_context(tc.tile_pool(name="wgt", bufs=2))
            p_hid = ctx.enter_context(tc.tile_pool(name="hid", bufs=2))
            p_tmp = ctx.enter_context(tc.tile_pool(name="tmp", bufs=2))
            p_sm = ctx.enter_context(tc.tile_pool(name="sm", bufs=2))
            p_cnd = ctx.enter_context(tc.tile_pool(name="cnd", bufs=1))
            p_cn = ctx.enter_context(tc.tile_pool(name="cn", bufs=1))
            p_ps = ctx.enter_context(tc.tile_pool(name="ps", bufs=8, space="PSUM"))

            # --- constants ---
            ident = p_cn.tile([128, 128], F32, tag="ident")
            make_identity(nc, ident[:])
            ident16 = p_cn.tile([128, 128], dt.float16, tag="ident16")
            make_identity(nc, ident16[:])
            cmask = p_cn.tile([128, 128], F32, tag="cmask")
            nc.sync.dma_start(cmask[:], cmask_d)
            ones_f = p_cn.tile([128, 1], F32, tag="onesf")
            nc.vector.memset(ones_f[:], 1.0)
            ones_rf = p_cn.tile([1, 128], F32, tag="onesrf")
            nc.vector.memset(ones_rf[:], 1.0)
            ones_col = p_cn.tile([128, 1], MMDT, tag="onesc")
            nc.vector.tensor_scalar(out=ones_col[:], in0=ones_f[:], scalar1=1.0,
                                    scalar2=None, op0=AL.mult)
            ones_row = p_cn.tile([1, 128], MMDT, tag="onesr")
            nc.vector.tensor_scalar(out=ones_row[:], in0=ones_rf[:], scalar1=1.0,
                                    scalar2=None, op0=AL.mult)
            ones_h = p_cn.tile([1, 128], MMDT, tag="onesh")
            nc.vector.tensor_scalar(out=ones_h[:], in0=ones_rf[:], scalar1=1.0 / H,
                                    scalar2=None, op0=AL.mult)
            eps_t = p_cn.tile([1, 1], F32, tag="eps")
            nc.vector.memset(eps_t[:], 1e-6)
            eps128 = p_cn.tile([128, 1], F32, tag="eps128")
            nc.vector.memset(eps128[:], 1e-6)

            embln_sb = p_cn.tile([128, 2, 4], F32, tag="embln")
            nc.sync.dma_start(embln_sb[:], embln.rearrange("s p m -> p s m"))
            lns = []
            for i in range(n_blocks):
                t = p_cn.tile([128, 2, 2, 4], F32, tag=f"ln{i}")
                nc.sync.dma_start(t[:], blk_w[i]['ln'].rearrange("l s p m -> p l s m"))
                lns.append(t)
            bqkvo = []
            for i in range(n_blocks):
                t = p_cn.tile([128, 4, 4], F32, tag=f"bq{i}")
                nc.sync.dma_start(t[:], blk_w[i]['qkvo_b'])
                bqkvo.append(t)
            bfc1 = []
            bfc2 = []
            for i in range(n_blocks):
                t1 = p_cn.tile([128, 16], F32, tag=f"b1{i}")
                nc.sync.dma_start(t1[:], blk_w[i]['fc1_b'])
                bfc1.append(t1)
                t2 = p_cn.tile([128, 4], F32, tag=f"b2{i}")
                nc.sync.dma_start(t2[:], blk_w[i]['fc2_b'])
                bfc2.append(t2)

            # ================= embeddings =================
            h = p_act.tile([128, KC, W2], MMDT, tag="act")

            # time-embedding gather: [T,H] rows per item
            te = []
            for it in range(BL):
                idx = p_sm.tile([128, 1], dt.int32, tag="idx")
                nc.sync.dma_start(idx[:], ts_idx[it * T:(it + 1) * T, :])
                g = p_tmp.tile([128, H], F32, tag="teg")
                nc.gpsimd.indirect_dma_start(
                    out=g[:], out_offset=None, in_=emb_tbl,
                    in_offset=bass.IndirectOffsetOnAxis(ap=idx[:, :1], axis=0))
                te.append(g)

            ew_s = p_cn.tile([SD, H], MMDT, tag="ews")
            nc.sync.dma_start(ew_s[:], ws_w.bitcast(MMDT))
            ew_a = p_cn.tile([AD, H], MMDT, tag="ewa")
            nc.sync.dma_start(ew_a[:], wa_w.bitcast(MMDT))
            ew_r = p_cn.tile([1, H], MMDT, tag="ewr")
            nc.sync.dma_start(ew_r[:], wr_w.bitcast(MMDT))
            eb = p_cn.tile([128, 3, 4], F32, tag="eb")
            nc.sync.dma_start(eb[:], emb_b.rearrange("s p m -> p s m"))

            xin = p_cn.tile([SD, BL * T], MMDT, tag="xs")
            nc.sync.dma_start(xin[:], states_t.bitcast(MMDT))
            ain = p_cn.tile([AD, BL * T], MMDT, tag="xa")
            nc.sync.dma_start(ain[:], actions_t.bitcast(MMDT))
            rin = p_cn.tile([1, BL * T], MMDT, tag="xr")
            nc.sync.dma_start(rin[:], rtg_t.bitcast(MMDT))

            streams = [(0, ew_r, rin, 1), (1, ew_s, xin, SD), (2, ew_a, ain, AD)]
            for off, wtile, xtile, kdim in streams:
                for mc in range(KC):
                    pe = p_ps.tile([128, BL * T], F32, tag="ps")
                    mm(pe[:], wtile[:kdim, mc * 128:(mc + 1) * 128], xtile[:kdim, :],
                       start=True, stop=False)
                    for it in range(BL):
                        nc.tensor.matmul(
                            pe[:, it * T:(it + 1) * T],
                            te[it][:, mc * 128:(mc + 1) * 128], ident[:],
                            is_transpose=True, start=False, stop=(it == BL - 1))
                    # h[:, mc, off::3] covers (item,t) in order
                    nc.vector.tensor_scalar(
                        out=h[:, mc, off::3], in0=pe[:],
                        scalar1=eb[:, off, mc:mc + 1], scalar2=None, op0=AL.add)

            # ================= layernorm helper =================
            def layernorm(x, sc_ap_fn, bi_ap_fn):
                """In-place LN over partition-H on x [128, KC, W2]."""
                xsq = p_hid.tile([128, 4, W2], MMDT, tag="hid")
                for kc in range(KC):
                    nc.scalar.activation(xsq[:, kc, :], x[:, kc, :], AF.Square)
                for it in range(BL):
                    sl = slice(it * S, (it + 1) * S)
                    s1 = p_ps.tile([1, S], F32, tag="ps")
                    s2 = p_ps.tile([1, S], F32, tag="ps")
                    for kc in range(KC):
                        mm(s1[:], ones_col[:, :1], x[:, kc, sl], start=(kc == 0), stop=(kc == KC - 1))
                    for kc in range(KC):
                        mm(s2[:], ones_col[:, :1], xsq[:, kc, sl], start=(kc == 0), stop=(kc == KC - 1))
                    # broadcast stats to 128 partitions, then do all math wide
                    s1c = p_sm.tile([1, S], MMDT, tag="s1c")
                    nc.scalar.copy(s1c[:, :], s1[:])
                    s2c = p_sm.tile([1, S], MMDT, tag="s2c")
                    nc.scalar.copy(s2c[:, :], s2[:])
                    mb = p_ps.tile([128, S], F32, tag="ps")
                    mm(mb[:], ones_h[:1, :], s1c[:, :], start=True, stop=True)   # mean bcast
                    sb2 = p_ps.tile([128, S], F32, tag="ps")
                    mm(sb2[:], ones_row[:1, :], s2c[:, :], start=True, stop=True)  # sumsq bcast
                    msqb = p_sm.tile([128, S], F32, tag="msqb")
                    nc.scalar.activation(msqb[:], mb[:], AF.Square)
                    ub = p_sm.tile([128, S], F32, tag="ub")
                    nc.vector.scalar_tensor_tensor(out=ub[:], in0=sb2[:], scalar=1.0 / H,
                                                   in1=msqb[:], op0=AL.mult, op1=AL.subtract)
                    sdb = p_sm.tile([128, S], F32, tag="sdb")
                    nc.scalar.activation(sdb[:], ub[:], AF.Sqrt, bias=eps128[:, :1])
                    rstd = p_sm.tile([128, S], F32, tag="rstd")
                    nc.vector.reciprocal(rstd[:], sdb[:])
                    for kc in range(KC):
                        nc.vector.tensor_tensor(out=x[:, kc, sl], in0=x[:, kc, sl],
                                                in1=mb[:], op=AL.subtract)
                        nc.vector.tensor_tensor(out=x[:, kc, sl], in0=x[:, kc, sl],
                                                in1=rstd[:], op=AL.mult)
                        nc.vector.tensor_scalar(out=x[:, kc, sl], in0=x[:, kc, sl],
                                                scalar1=sc_ap_fn(kc), scalar2=bi_ap_fn(kc),
                                                op0=AL.mult, op1=AL.add)

            layernorm(h, lambda kc: embln_sb[:, 0, kc:kc + 1], lambda kc: embln_sb[:, 1, kc:kc + 1])

            # ================= transformer blocks =================
            for bi in range(n_blocks):
                bw = blk_w[bi]
                bb = bqkvo[bi]
                # --- q/k projections (transposed layout) ---
                qt = p_qk.tile([128, KC, W2], MMDT, tag="qk")
                kt = p_qk.tile([128, KC, W2], MMDT, tag="qk")
                for pi, dst in ((0, qt), (1, kt)):
                    wch = p_wgt.tile([128, KC, H], MMDT, tag="wgt")
                    nc.sync.dma_start(wch[:], bw['qkv'][:, pi].bitcast(MMDT))
                    for mc in range(KC):
                        for it in range(BL):
                            pp = p_ps.tile([128, S], F32, tag="ps")
                            for kc in range(KC):
                                mm(pp[:], wch[:, kc, mc * 128:(mc + 1) * 128],
                                   h[:, kc, it * S:(it + 1) * S],
                                   start=(kc == 0), stop=(kc == KC - 1))
                            nc.vector.tensor_scalar(
                                out=dst[:, mc, it * S:(it + 1) * S], in0=pp[:],
                                scalar1=bb[:, pi, mc:mc + 1], scalar2=None, op0=AL.add)
                # --- v in natural layout [tok, H] ---
                vch = p_wgt.tile([128, KC, H], MMDT, tag="wgt")
                nc.sync.dma_start(vch[:], bw['qkv'][:, 2].bitcast(MMDT))
                vn = p_vn.tile([128, BL * 3, H], MMDT, tag="vn")
                for it in range(BL):
                    for tt in range(3):
                        pp = p_ps.tile([128, H], F32, tag="ps")
                        for kc in range(KC):
                            mm(pp[:], h[:, kc, it * S + tt * 128: it * S + (tt + 1) * 128],
                               vch[:, kc, :], start=(kc == 0), stop=(kc == KC - 1))
                        nc.scalar.copy(vn[:, it * 3 + tt, :], pp[:])
                # --- attention per item/head ---
                aot = p_qk.tile([128, KC, W2], MMDT, tag="qk")
                for it in range(BL):
                    for hp in range(NH // 2):
                        ao_ps = []
                        for sub in range(2):
                            hd = hp * 2 + sub
                            kc_h = hd // 2
                            pb = 64 * (hd & 1)
                            q_ap = qt[pb:pb + 64, kc_h, it * S:(it + 1) * S]
                            k_ap = kt[pb:pb + 64, kc_h, it * S:(it + 1) * S]
                            wsb = p_w.tile([128, 3, S], dt.float16, tag="wsb")
                            nc.gpsimd.memset(wsb[:, 0, 128:S], 0.0)
                            nc.gpsimd.memset(wsb[:, 1, 256:S], 0.0)
                            rs = p_sm.tile([128, 3], F32, tag="rs")
                            for tt in range(3):
                                span = (tt + 1) * 128
                                sc = p_ps.tile([128, S], F32, tag="ps")
                                mm(sc[:, :span], q_ap[:, tt * 128:(tt + 1) * 128],
                                   k_ap[:, :span], start=True, stop=True)
                                nc.vector.tensor_tensor(
                                    out=sc[:, tt * 128:span], in0=sc[:, tt * 128:span],
                                    in1=cmask[:], op=AL.add)
                                nc.scalar.activation(wsb[:, tt, :span], sc[:, :span],
                                                     AF.Exp, accum_out=rs[:, tt:tt + 1])
                            rr = p_sm.tile([128, 3], F32, tag="rr")
                            nc.vector.reciprocal(rr[:], rs[:])
                            for tt in range(3):
                                span = (tt + 1) * 128
                                nc.vector.tensor_scalar(
                                    out=wsb[:, tt, :span], in0=wsb[:, tt, :span],
                                    scalar1=rr[:, tt:tt + 1], scalar2=None, op0=AL.mult)
                            nc.sync.dma_start(
                                attns[bi, it, hd].rearrange("(c p) f -> p c f", p=128),
                                wsb[:])
                            # transpose w -> wT tiles
                            wt = p_wt.tile([128, 3, S], MMDT, tag="wtsb")
                            for ft in range(3):
                                tspan = S - ft * 128
                                tp = p_ps.tile([128, S], dt.float16, tag="ps")
                                for tt in range(ft, 3):
                                    nc.tensor.matmul(
                                        tp[:, (tt - ft) * 128:(tt - ft + 1) * 128],
                                        wsb[:, tt, ft * 128:(ft + 1) * 128],
                                        ident16[:], is_transpose=True,
                                        start=(tt == ft), stop=(tt == 2))
                                nc.vector.tensor_copy(wt[:, ft, :tspan], tp[:, :tspan])
                            # attn @ v -> [64, S] in psum (col-packed pairs)
                            ap_ = p_ps.tile([128, S], F32, tag="ps")
                            ao_ps.append((ap_, wt))
                            for ft in range(3):
                                tspan = S - ft * 128
                                mm(ap_[pb:pb + 64, ft * 128:S],
                                   vn[:, it * 3 + ft, hd * 64:(hd + 1) * 64],
                                   wt[:, ft, :tspan],
                                   start=(ft == 0), stop=(ft == 2),
                                   tile_position=(0, pb))
                        # evacuate pair into aot with v-bias fold
                        for sub in range(2):
                            hd = hp * 2 + sub
                            ap_, _ = ao_ps[sub]
                            pb = 64 * (hd & 1)
                            nc.vector.tensor_scalar(
                                out=aot[pb:pb + 64, hd // 2, it * S:(it + 1) * S],
                                in0=ap_[pb:pb + 64, :],
                                scalar1=bb[pb:pb + 64, 2, (hd // 2):(hd // 2) + 1],
                                scalar2=None, op0=AL.add)
                # --- output projection + residual ---
                och = p_wgt.tile([128, KC, H], MMDT, tag="wgt")
                nc.sync.dma_start(och[:], bw['o'].bitcast(MMDT))
                h2 = p_act.tile([128, KC, W2], MMDT, tag="act")
                for mc in range(KC):
                    for it in range(BL):
                        pp = p_ps.tile([128, S], F32, tag="ps")
                        for kc in range(KC):
                            mm(pp[:], och[:, kc, mc * 128:(mc + 1) * 128],
                               aot[:, kc, it * S:(it + 1) * S],
                               start=(kc == 0), stop=(kc == KC - 1))
                        nc.vector.scalar_tensor_tensor(
                            out=h2[:, mc, it * S:(it + 1) * S], in0=pp[:],
                            scalar=bb[:, 3, mc:mc + 1],
                            in1=h[:, mc, it * S:(it + 1) * S],
                            op0=AL.add, op1=AL.add)
                h = h2
                ln = lns[bi]
                layernorm(h, lambda kc: ln[:, 0, 0, kc:kc + 1], lambda kc: ln[:, 0, 1, kc:kc + 1])
                # --- ffn ---
                h3 = p_act.tile([128, KC, W2], MMDT, tag="act")
                b1 = bfc1[bi]
                for hc in range(4):
                    f1 = p_wgt.tile([128, KC, H], MMDT, tag="wgt")
                    nc.sync.dma_start(f1[:], bw['fc1'][:, :, hc * 512:(hc + 1) * 512].bitcast(MMDT))
                    hid = p_hid.tile([128, 4, W2], MMDT, tag="hid")
                    for hm in range(4):
                        for it in range(BL):
                            pp = p_ps.tile([128, S], F32, tag="ps")
                            for kc in range(KC):
                                mm(pp[:], f1[:, kc, hm * 128:(hm + 1) * 128],
                                   h[:, kc, it * S:(it + 1) * S],
                                   start=(kc == 0), stop=(kc == KC - 1))
                            nc.scalar.activation(
                                hid[:, hm, it * S:(it + 1) * S], pp[:],
                                AF.Gelu_apprx_tanh,
                                bias=b1[:, hc * 4 + hm:hc * 4 + hm + 1])
                    f2 = p_wgt.tile([128, 4, H], MMDT, tag="wgt")
                    nc.sync.dma_start(f2[:], bw['fc2'][:, hc * 4:(hc + 1) * 4, :].bitcast(MMDT))
                    for mc in range(KC):
                        for it in range(BL):
                            pp2 = p_ps.tile([128, S], F32, tag="ps")
                            for kk in range(4):
                                mm(pp2[:], f2[:, kk, mc * 128:(mc + 1) * 128],
                                   hid[:, kk, it * S:(it + 1) * S],
                                   start=(kk == 0), stop=(kk == 3))
                            if hc == 0:
                                # h3 = x1 + psum + b2 (first chunk: include residual+bias)
                                nc.vector.scalar_tensor_tensor(
                                    out=h3[:, mc, it * S:(it + 1) * S], in0=pp2[:],
                                    scalar=bfc2[bi][:, mc:mc + 1],
                                    in1=h[:, mc, it * S:(it + 1) * S],
                                    op0=AL.add, op1=AL.add)
                            else:
                                nc.vector.tensor_tensor(
                                    out=h3[:, mc, it * S:(it + 1) * S],
                                    in0=h3[:, mc, it * S:(it + 1) * S],
                                    in1=pp2[:], op=AL.add)
                h = h3
                layernorm(h, lambda kc: ln[:, 1, 0, kc:kc + 1], lambda kc: ln[:, 1, 1, kc:kc + 1])

            if debug_h:
                nc.sync.dma_start(hdbg, h[:])

            # ================= tail: heads + align attention =================
            if tail:
                def rep_ap(kc, off, it=None):
                    """strided stream columns; it=None -> both items [128, 2, T]"""
                    if it is None:
                        return h[:, kc, :].rearrange("p (i t) -> p i t", i=BL)[:, :, off::3]
                    return h[:, kc, it * S + off:(it + 1) * S:3]

                # prediction heads from action_repr (off=2) and state_repr (off=1)
                for wt_, bt_, od, outdim, off, act in (
                        (ps_w, ps_b, sp_o, SD, 2, None),
                        (pr_w, pr_b, rp_o, 1, 2, None),
                        (pa_w, pa_b, ap_o, AD, 1, AF.Tanh)):
                    wsb_ = p_cn.tile([128, KC, outdim], MMDT, tag=f"hw{outdim}_{off}")
                    nc.sync.dma_start(wsb_[:], wt_.bitcast(MMDT))
                    bsb_ = p_cn.tile([outdim, 1], F32, tag=f"hb{outdim}_{off}")
                    nc.sync.dma_start(bsb_[:], bt_)
                    for it in range(BL):
                        pp = p_ps.tile([outdim, T], F32, tag="ps")
                        for kc in range(KC):
                            mm(pp[:], wsb_[:, kc, :], rep_ap(kc, off, it),
                               start=(kc == 0), stop=(kc == KC - 1))
                        ot = p_sm.tile([outdim, T], F32, tag=f"ho{outdim}_{off}")
                        if act is None:
                            nc.vector.tensor_scalar(out=ot[:], in0=pp[:],
                                                    scalar1=bsb_[:, :1], scalar2=None, op0=AL.add)
                        else:
                            nc.scalar.activation(ot[:], pp[:], act, bias=bsb_[:, :1])
                        nc.sync.dma_start(od[it], ot[:])

                # --- align attention (q from state/action reprs, kv from return repr) ---
                ab = p_cn.tile([128, 4, 4], F32, tag="alb")
                nc.sync.dma_start(ab[:], al_qkvo_b)
                # k,v,qs,qa transposed [128, KC, 2, T]
                kt_a = p_qk.tile([128, KC, W2], MMDT, tag="qk")
                q_s = p_qk.tile([128, KC, W2], MMDT, tag="qk")
                q_a = p_qk.tile([128, KC, W2], MMDT, tag="qk")

                def proj_t(dst, wch, pi, off):
                    for mc in range(KC):
                        pp = p_ps.tile([128, BL * T], F32, tag="ps")
                        for kc in range(KC):
                            mm(pp[:], wch[:, kc, mc * 128:(mc + 1) * 128], rep_ap(kc, off),
                               start=(kc == 0), stop=(kc == KC - 1))
                        nc.vector.tensor_scalar(
                            out=dst[:, mc, :BL * T], in0=pp[:],
                            scalar1=ab[:, pi, mc:mc + 1], scalar2=None, op0=AL.add)

                aw = p_wgt.tile([128, KC, H], MMDT, tag="wgt")
                nc.sync.dma_start(aw[:], al_qkv_w[:, 0].bitcast(MMDT))
                proj_t(q_s, aw, 0, 1)
                proj_t(q_a, aw, 0, 2)
                aw2 = p_wgt.tile([128, KC, H], MMDT, tag="wgt")
                nc.sync.dma_start(aw2[:], al_qkv_w[:, 1].bitcast(MMDT))
                proj_t(kt_a, aw2, 1, 0)
                # v natural per item [T, H]
                aw3 = p_wgt.tile([128, KC, H], MMDT, tag="wgt")
                nc.sync.dma_start(aw3[:], al_qkv_w[:, 2].bitcast(MMDT))
                vn_a = p_vn.tile([128, BL * 3, H], MMDT, tag="vn")
                for it in range(BL):
                    pp = p_ps.tile([128, H], F32, tag="ps")
                    for kc in range(KC):
                        mm(pp[:], rep_ap(kc, 0, it), aw3[:, kc, :],
                           start=(kc == 0), stop=(kc == KC - 1))
                    nc.scalar.copy(vn_a[:, it * 3, :], pp[:])

                ow = p_wgt.tile([128, KC, H], MMDT, tag="wgt")
                nc.sync.dma_start(ow[:], al_o_w.bitcast(MMDT))
                pw = p_wgt.tile([128, KC, H], MMDT, tag="wgt")
                nc.sync.dma_start(pw[:], al_p_w.bitcast(MMDT))
                apb = p_cn.tile([128, 4], F32, tag="apb")
                nc.sync.dma_start(apb[:], al_p_b)

                for qsrc, od in ((q_s, sproj_o), (q_a, aproj_o)):
                    cnd = p_cnd.tile([128, KC, W2], F32, tag="cnd")
                    for it in range(BL):
                        # attention: heads
                        aot2 = p_w.tile([128, KC, T], MMDT, tag="alao")
                        for hd in range(NH):
                            kc_h = hd // 2
                            pb = 64 * (hd & 1)
                            q_ap = qsrc[pb:pb + 64, kc_h, it * T:(it + 1) * T]
                            k_ap = kt_a[pb:pb + 64, kc_h, it * T:(it + 1) * T]
                            sc = p_ps.tile([128, T], F32, tag="ps")
                            mm(sc[:], q_ap, k_ap, start=True, stop=True)
                            nc.vector.tensor_tensor(out=sc[:], in0=sc[:], in1=cmask[:], op=AL.add)
                            u = p_sm.tile([128, T], dt.float16, tag="alu")
                            rs = p_sm.tile([128, 1], F32, tag="alrs")
                            nc.scalar.activation(u[:], sc[:], AF.Exp, accum_out=rs[:, :1])
                            rr = p_sm.tile([128, 1], F32, tag="alrr")
                            nc.vector.reciprocal(rr[:], rs[:])
                            nc.vector.tensor_scalar(out=u[:], in0=u[:], scalar1=rr[:, :1],
                                                    scalar2=None, op0=AL.mult)
                            tp = p_ps.tile([128, T], dt.float16, tag="ps")
                            nc.tensor.matmul(tp[:], u[:], ident16[:],
                                             is_transpose=True, start=True, stop=True)
                            ut = p_sm.tile([128, T], MMDT, tag="alut")
                            nc.vector.tensor_copy(ut[:], tp[:])
                            av = p_ps.tile([128, T], F32, tag="ps")
                            mm(av[pb:pb + 64, :], vn_a[:, it * 3, hd * 64:(hd + 1) * 64],
                               ut[:], start=True, stop=True, tile_position=(0, pb))
                            nc.vector.tensor_scalar(
                                out=aot2[pb:pb + 64, kc_h, :], in0=av[pb:pb + 64, :],
                                scalar1=ab[pb:pb + 64, 2, kc_h:kc_h + 1],
                                scalar2=None, op0=AL.add)
                        # o proj -> cond, then align_proj -> cnd
                        condt = p_wt.tile([128, KC, T], MMDT, tag="alcond")
                        for mc in range(KC):
                            pp = p_ps.tile([128, T], F32, tag="ps")
                            for kc in range(KC):
                                mm(pp[:], ow[:, kc, mc * 128:(mc + 1) * 128], aot2[:, kc, :],
                                   start=(kc == 0), stop=(kc == KC - 1))
                            nc.vector.tensor_scalar(
                                out=condt[:, mc, :], in0=pp[:],
                                scalar1=ab[:, 3, mc:mc + 1], scalar2=None, op0=AL.add)
                        for mc in range(KC):
                            pp = p_ps.tile([128, T], F32, tag="ps")
                            for kc in range(KC):
                                mm(pp[:], pw[:, kc, mc * 128:(mc + 1) * 128], condt[:, kc, :],
                                   start=(kc == 0), stop=(kc == KC - 1))
                            nc.vector.tensor_scalar(
                                out=cnd[:, mc, it * T:it * T + T], in0=pp[:],
                                scalar1=apb[:, mc:mc + 1], scalar2=None, op0=AL.add)
                    # l2 normalize over H and write out
                    csq = p_hid.tile([128, KC, W2], MMDT, tag="hid")
                    for kc in range(KC):
                        nc.scalar.activation(csq[:, kc, :BL * T], cnd[:, kc, :BL * T], AF.Square)
                    s2 = p_ps.tile([1, BL * T], F32, tag="ps")
                    for kc in range(KC):
                        mm(s2[:], ones_col[:, :1], csq[:, kc, :BL * T],
                           start=(kc == 0), stop=(kc == KC - 1))
                    nrm = p_sm.tile([1, BL * T], F32, tag="nrm")
                    nc.scalar.activation(nrm[:], s2[:], AF.Sqrt)
                    nc.vector.tensor_scalar(out=nrm[:], in0=nrm[:], scalar1=1e-8,
                                            scalar2=None, op0=AL.add)
                    rn = p_sm.tile([1, BL * T], MMDT, tag="rn")
                    nc.vector.reciprocal(rn[:], nrm[:])
                    nb_ = p_ps.tile([128, BL * T], F32, tag="ps")
                    mm(nb_[:], ones_row[:1, :], rn[:1, :], start=True, stop=True)
                    for kc in range(KC):
                        nc.vector.tensor_tensor(out=cnd[:, kc, :BL * T], in0=cnd[:, kc, :BL * T],
                                                in1=nb_[:], op=AL.mult)
                    for it in range(BL):
                        nc.sync.dma_start(
                            od[it].rearrange("(c p) t -> p c t", p=128),
                            cnd[:, :, it * T:(it + 1) * T])

    nc.compile()
    return nc


def _get(key, **kw):
    if key not in _BUILT:
        _BUILT[key] = _build(**kw)
    return _BUILT[key]


# ----------------------------------------------------------------------------
# entry point
# ----------------------------------------------------------------------------

def kernel(timesteps, states, actions, returns_to_go, params, _trace=False, _tmpdir=None):
    from concourse.bass_utils import run_bass_kernel_spmd

    nc = _get('full')
    in_maps = _pack_inputs(timesteps, states, actions, returns_to_go, params)
    res = run_bass_kernel_spmd(nc, in_maps, list(range(NC)), trace=_trace, tmpdir=_tmpdir)
    kernel._last = res

    outs = res.results
    attns = np.concatenate([o['attns'] for o in outs], axis=1).astype(np.float32)
    sp = np.concatenate([o['sp'] for o in outs], 0).transpose(0, 2, 1)  # [16,128,17]
    ap_ = np.concatenate([o['ap_'] for o in outs], 0).transpose(0, 2, 1)
    rp = np.concatenate([o['rp'] for o in outs], 0).transpose(0, 2, 1)
    sproj = np.concatenate([o['sproj'] for o in outs], 0).transpose(0, 2, 1)
    aproj = np.concatenate([o['aproj'] for o in outs], 0).transpose(0, 2, 1)
    return sp, ap_, rp, attns, (sproj, aproj)


# revision 21
# speedup vs baseline: 1.0250x; 1.0250x over previous
"""Decision Transformer Bass kernel for 8 Trainium2 NeuronCores.

Sharding: data-parallel over batch B=16 -> 2 items per core, params replicated.
All activations live transposed in SBUF: [H partitions, token cols].
Matmuls use float32r views (full PE rate at N>=256, fp32 numerics).
"""

import numpy as np

# model dims (hardcoded per contract)
B, T, SD, AD = 16, 128, 17, 6
H, NH, NB, MAXTS = 512, 8, 6, 4096
D = H // NH          # 64
S = 3 * T            # 384 tokens per item
NC = 8               # cores
BL = B // NC         # 2 items per core
W2 = BL * S          # 768 activation cols per core
KC = H // 128        # 4 k-chunks of hidden dim
FF = 4 * H           # 2048
NEG = -1.0e30
MM_F16 = True   # fp16 matmul operands (fast path); False -> float32r

_BUILT = {}


# ----------------------------------------------------------------------------
# host-side weight packing
# ----------------------------------------------------------------------------

def _lhsT(w):
    """[Kin, M] -> [128, Kin//128, M] chunk layout for lhsT slices."""
    w = np.ascontiguousarray(np.asarray(w, np.float32))
    kin, m = w.shape
    kc = kin // 128
    return np.ascontiguousarray(w.reshape(kc, 128, m).transpose(1, 0, 2))


def _bcol(b, mc=4):
    """[M] bias -> [128, mc] per-partition layout."""
    b = np.asarray(b, np.float32)
    return np.ascontiguousarray(b.reshape(mc, 128).T)


def _pack_inputs(timesteps, states, actions, returns_to_go, params):
    """Returns (shared weight map, list of per-core input maps)."""
    p = params
    shared = {}
    scale_q = 1.0 / np.sqrt(np.float32(D))

    for i in range(NB):
        bp = p['blocks'][i]
        a = bp['attn']
        qkv = np.stack([_lhsT(np.asarray(a['q']['w']) * scale_q),
                        _lhsT(a['k']['w']), _lhsT(a['v']['w'])], axis=1)
        shared[f'qkv_w_{i}'] = np.ascontiguousarray(qkv)          # [128,3,4,512]
        shared[f'o_w_{i}'] = _lhsT(a['o']['w'])                   # [128,4,512]
        qb = np.stack([_bcol(np.asarray(a['q']['b']) * scale_q),
                       _bcol(a['k']['b']), _bcol(a['v']['b']),
                       _bcol(a['o']['b'])], axis=1)
        shared[f'qkvo_b_{i}'] = np.ascontiguousarray(qb)          # [128,4,4]
        shared[f'fc1_w_{i}'] = _lhsT(bp['fc1']['w'])              # [128,4,2048]
        shared[f'fc1_b_{i}'] = _bcol(bp['fc1']['b'], 16)          # [128,16]
        shared[f'fc2_w_{i}'] = _lhsT(bp['fc2']['w'])              # [128,16,512]
        shared[f'fc2_b_{i}'] = _bcol(bp['fc2']['b'])              # [128,4]
        ln = np.stack([np.stack([_bcol(bp['ln1']['scale']), _bcol(bp['ln1']['bias'])], 0),
                       np.stack([_bcol(bp['ln2']['scale']), _bcol(bp['ln2']['bias'])], 0)], 0)
        shared[f'ln_{i}'] = np.ascontiguousarray(ln)              # [128,2,2,4] -> idx [p? see below]

    shared['embln'] = np.ascontiguousarray(
        np.stack([_bcol(p['embed_ln']['scale']), _bcol(p['embed_ln']['bias'])], 0))  # [2,128,4]
    shared['emb_tbl'] = np.ascontiguousarray(np.asarray(p['embed_timestep'], np.float32))
    shared['ws_w'] = np.ascontiguousarray(np.asarray(p['embed_state']['w'], np.float32))   # [17,512]
    shared['wa_w'] = np.ascontiguousarray(np.asarray(p['embed_action']['w'], np.float32))  # [6,512]
    shared['wr_w'] = np.ascontiguousarray(np.asarray(p['embed_rtg']['w'], np.float32))     # [1,512]
    emb_b = np.stack([_bcol(p['embed_rtg']['b']), _bcol(p['embed_state']['b']),
                      _bcol(p['embed_action']['b'])], 0)
    shared['emb_b'] = np.ascontiguousarray(emb_b)                 # [3,128,4] (r,s,a)

    aa = p['align_attn']
    aqkv = np.stack([_lhsT(np.asarray(aa['q']['w']) * scale_q),
                     _lhsT(aa['k']['w']), _lhsT(aa['v']['w'])], axis=1)
    shared['al_qkv_w'] = np.ascontiguousarray(aqkv)
    shared['al_o_w'] = _lhsT(aa['o']['w'])
    aqb = np.stack([_bcol(np.asarray(aa['q']['b']) * scale_q), _bcol(aa['k']['b']),
                    _bcol(aa['v']['b']), _bcol(aa['o']['b'])], axis=1)
    shared['al_qkvo_b'] = np.ascontiguousarray(aqb)               # [128,4,4]
    shared['al_p_w'] = _lhsT(p['align_proj']['w'])                # [128,4,512]
    shared['al_p_b'] = _bcol(p['align_proj']['b'])                # [128,4]

    shared['ps_w'] = _lhsT(p['predict_state']['w'])               # [128,4,17]
    shared['pa_w'] = _lhsT(p['predict_action']['w'])              # [128,4,6]
    shared['pr_w'] = _lhsT(p['predict_rtg']['w'])                 # [128,4,1]
    shared['ps_b'] = np.ascontiguousarray(np.asarray(p['predict_state']['b'], np.float32).reshape(SD, 1))
    shared['pa_b'] = np.ascontiguousarray(np.asarray(p['predict_action']['b'], np.float32).reshape(AD, 1))
    shared['pr_b'] = np.ascontiguousarray(np.asarray(p['predict_rtg']['b'], np.float32).reshape(1, 1))

    # additive causal mask for diagonal 128x128 tiles
    m = np.zeros((128, 128), np.float32)
    m[np.triu_indices(128, 1)] = NEG
    shared['cmask'] = m

    ts = np.asarray(timesteps).astype(np.int32)           # [16,128]
    st = np.asarray(states, np.float32)                   # [16,128,17]
    ac = np.asarray(actions, np.float32)
    rt = np.asarray(returns_to_go, np.float32)

    in_maps = []
    for c in range(NC):
        sl = slice(c * BL, (c + 1) * BL)
        im = dict(shared)
        im['ts_idx'] = np.ascontiguousarray(ts[sl].reshape(BL * T, 1))
        # [feat, item*T + t]
        im['states_t'] = np.ascontiguousarray(st[sl].reshape(BL * T, SD).T)
        im['actions_t'] = np.ascontiguousarray(ac[sl].reshape(BL * T, AD).T)
        im['rtg_t'] = np.ascontiguousarray(rt[sl].reshape(BL * T, 1).T)
        in_maps.append(im)
    if MM_F16:
        f16_names = (['al_qkv_w', 'al_o_w', 'al_p_w', 'ws_w', 'wa_w', 'wr_w',
                      'ps_w', 'pa_w', 'pr_w']
                     + [f'{n}_{i}' for i in range(NB) for n in ('qkv_w', 'o_w', 'fc1_w', 'fc2_w')])
        for nm in f16_names:
            shared[nm] = shared[nm].astype(np.float16)
        for im in in_maps:
            for nm in f16_names:
                im[nm] = shared[nm]
            for nm in ('states_t', 'actions_t', 'rtg_t'):
                im[nm] = im[nm].astype(np.float16)
    return in_maps


# ----------------------------------------------------------------------------
# device program
# ----------------------------------------------------------------------------

def _build(n_blocks=NB, tail=True, debug_h=False):
    import concourse.bass as bass
    import concourse.tile as tile
    from concourse import bacc, mybir
    from concourse.masks import make_identity

    dt = mybir.dt
    F32 = dt.float32
    F32R = dt.float32r
    MMDT = dt.float16 if MM_F16 else F32R
    WDT = dt.float16 if MM_F16 else F32
    AL = mybir.AluOpType
    AF = mybir.ActivationFunctionType

    nc = bacc.Bacc("TRN2", target_bir_lowering=False, debug=False)

    def din(name, shape, dty=F32):
        return nc.dram_tensor(name, list(shape), dty, kind="ExternalInput").ap()

    def dout(name, shape, dty=F32):
        return nc.dram_tensor(name, list(shape), dty, kind="ExternalOutput").ap()

    # --- dram io ---
    ts_idx = din('ts_idx', [BL * T, 1], dt.int32)
    states_t = din('states_t', [SD, BL * T], WDT)
    actions_t = din('actions_t', [AD, BL * T], WDT)
    rtg_t = din('rtg_t', [1, BL * T], WDT)
    emb_tbl = din('emb_tbl', [MAXTS, H])
    ws_w = din('ws_w', [SD, H], WDT); wa_w = din('wa_w', [AD, H], WDT); wr_w = din('wr_w', [1, H], WDT)
    emb_b = din('emb_b', [3, 128, 4])
    embln = din('embln', [2, 128, 4])
    cmask_d = din('cmask', [128, 128])
    blk_w = []
    for i in range(n_blocks):
        blk_w.append(dict(
            qkv=din(f'qkv_w_{i}', [128, 3, KC, H], WDT),
            o=din(f'o_w_{i}', [128, KC, H], WDT),
            qkvo_b=din(f'qkvo_b_{i}', [128, 4, 4]),
            fc1=din(f'fc1_w_{i}', [128, KC, FF], WDT),
            fc1_b=din(f'fc1_b_{i}', [128, 16]),
            fc2=din(f'fc2_w_{i}', [128, 16, H], WDT),
            fc2_b=din(f'fc2_b_{i}', [128, 4]),
            ln=din(f'ln_{i}', [2, 2, 128, 4]),
        ))
    # unused input names still must be declared if provided? we only pass what we declare.
    if tail:
        al_qkv_w = din('al_qkv_w', [128, 3, KC, H], WDT)
        al_o_w = din('al_o_w', [128, KC, H], WDT)
        al_qkvo_b = din('al_qkvo_b', [128, 4, 4])
        al_p_w = din('al_p_w', [128, KC, H], WDT)
        al_p_b = din('al_p_b', [128, 4])
        ps_w = din('ps_w', [128, KC, SD], WDT); ps_b = din('ps_b', [SD, 1])
        pa_w = din('pa_w', [128, KC, AD], WDT); pa_b = din('pa_b', [AD, 1])
        pr_w = din('pr_w', [128, KC, 1], WDT); pr_b = din('pr_b', [1, 1])

    attns = dout('attns', [n_blocks, BL, NH, S, S], dt.float16) if n_blocks else None
    if tail:
        sp_o = dout('sp', [BL, SD, T])
        ap_o = dout('ap_', [BL, AD, T])
        rp_o = dout('rp', [BL, 1, T])
        sproj_o = dout('sproj', [BL, H, T])
        aproj_o = dout('aproj', [BL, H, T])
    if debug_h:
        hdbg = dout('hdbg', [128, KC, W2], MMDT)

    def mm(out, lhsT, rhs, **kw):
        nc.tensor.matmul(out, lhsT, rhs, **kw)

    with tile.TileContext(nc) as tc:
        import contextlib
        ctx = contextlib.ExitStack()
        with ctx:
            ctx.enter_context(nc.allow_low_precision(reason="f32r matmul operands (tf32-like, intended)"))
            p_act = ctx.enter_context(tc.tile_pool(name="act", bufs=3))
            p_qk = ctx.enter_context(tc.tile_pool(name="qk", bufs=3))
            p_vn = ctx.enter_context(tc.tile_pool(name="vn", bufs=2))
            p_w = ctx.enter_context(tc.tile_pool(name="wsb", bufs=6))
            p_wt = ctx.enter_context(tc.tile_pool(name="wtsb", bufs=6))
            p_wgt = ctx.enter_context(tc.tile_pool(name="wgt", bufs=4))
            p_hid = ctx.enter_context(tc.tile_pool(name="hid", bufs=3))
            p_tmp = ctx.enter_context(tc.tile_pool(name="tmp", bufs=2))
            p_sm = ctx.enter_context(tc.tile_pool(name="sm", bufs=3))
            p_cnd = ctx.enter_context(tc.tile_pool(name="cnd", bufs=1))
            p_cn = ctx.enter_context(tc.tile_pool(name="cn", bufs=1))
            p_ps = ctx.enter_context(tc.tile_pool(name="ps", bufs=8, space="PSUM"))

            # --- constants ---
            ident = p_cn.tile([128, 128], F32, tag="ident")
            make_identity(nc, ident[:])
            ident16 = p_cn.tile([128, 128], dt.float16, tag="ident16")
            make_identity(nc, ident16[:])
            cmask = p_cn.tile([128, 128], F32, tag="cmask")
            nc.sync.dma_start(cmask[:], cmask_d)
            ones_f = p_cn.tile([128, 1], F32, tag="onesf")
            nc.vector.memset(ones_f[:], 1.0)
            ones_rf = p_cn.tile([1, 128], F32, tag="onesrf")
            nc.vector.memset(ones_rf[:], 1.0)
            ones_col = p_cn.tile([128, 1], MMDT, tag="onesc")
            nc.vector.tensor_scalar(out=ones_col[:], in0=ones_f[:], scalar1=1.0,
                                    scalar2=None, op0=AL.mult)
            ones_row = p_cn.tile([1, 128], MMDT, tag="onesr")
            nc.vector.tensor_scalar(out=ones_row[:], in0=ones_rf[:], scalar1=1.0,
                                    scalar2=None, op0=AL.mult)
            ones_h = p_cn.tile([1, 128], MMDT, tag="onesh")
            nc.vector.tensor_scalar(out=ones_h[:], in0=ones_rf[:], scalar1=1.0 / H,
                                    scalar2=None, op0=AL.mult)
            eps_t = p_cn.tile([1, 1], F32, tag="eps")
            nc.vector.memset(eps_t[:], 1e-6)
            eps128 = p_cn.tile([128, 1], F32, tag="eps128")
            nc.vector.memset(eps128[:], 1e-6)
            eps8_t = p_cn.tile([1, 1], F32, tag="eps8")
            nc.vector.memset(eps8_t[:], 1e-8)

            embln_sb = p_cn.tile([128, 2, 4], F32, tag="embln")
            nc.sync.dma_start(embln_sb[:], embln.rearrange("s p m -> p s m"))
            lns = []
            for i in range(n_blocks):
                t = p_cn.tile([128, 2, 2, 4], F32, tag=f"ln{i}")
                nc.sync.dma_start(t[:], blk_w[i]['ln'].rearrange("l s p m -> p l s m"))
                lns.append(t)
            bqkvo = []
            for i in range(n_blocks):
                t = p_cn.tile([128, 4, 4], F32, tag=f"bq{i}")
                nc.sync.dma_start(t[:], blk_w[i]['qkvo_b'])
                bqkvo.append(t)
            bfc1 = []
            bfc2 = []
            for i in range(n_blocks):
                t1 = p_cn.tile([128, 16], F32, tag=f"b1{i}")
                nc.sync.dma_start(t1[:], blk_w[i]['fc1_b'])
                bfc1.append(t1)
                t2 = p_cn.tile([128, 4], F32, tag=f"b2{i}")
                nc.sync.dma_start(t2[:], blk_w[i]['fc2_b'])
                bfc2.append(t2)

            # ================= embeddings =================
            h = p_act.tile([128, KC, W2], MMDT, tag="act")

            # time-embedding gather: [T,H] rows per item
            te = []
            for it in range(BL):
                idx = p_sm.tile([128, 1], dt.int32, tag="idx")
                nc.sync.dma_start(idx[:], ts_idx[it * T:(it + 1) * T, :])
                g = p_tmp.tile([128, H], F32, tag="teg")
                nc.gpsimd.indirect_dma_start(
                    out=g[:], out_offset=None, in_=emb_tbl,
                    in_offset=bass.IndirectOffsetOnAxis(ap=idx[:, :1], axis=0))
                te.append(g)

            ew_s = p_cn.tile([SD, H], MMDT, tag="ews")
            nc.sync.dma_start(ew_s[:], ws_w.bitcast(MMDT))
            ew_a = p_cn.tile([AD, H], MMDT, tag="ewa")
            nc.sync.dma_start(ew_a[:], wa_w.bitcast(MMDT))
            ew_r = p_cn.tile([1, H], MMDT, tag="ewr")
            nc.sync.dma_start(ew_r[:], wr_w.bitcast(MMDT))
            eb = p_cn.tile([128, 3, 4], F32, tag="eb")
            nc.sync.dma_start(eb[:], emb_b.rearrange("s p m -> p s m"))

            xin = p_cn.tile([SD, BL * T], MMDT, tag="xs")
            nc.sync.dma_start(xin[:], states_t.bitcast(MMDT))
            ain = p_cn.tile([AD, BL * T], MMDT, tag="xa")
            nc.sync.dma_start(ain[:], actions_t.bitcast(MMDT))
            rin = p_cn.tile([1, BL * T], MMDT, tag="xr")
            nc.sync.dma_start(rin[:], rtg_t.bitcast(MMDT))

            streams = [(0, ew_r, rin, 1), (1, ew_s, xin, SD), (2, ew_a, ain, AD)]
            for off, wtile, xtile, kdim in streams:
                for mc in range(KC):
                    pe = p_ps.tile([128, BL * T], F32, tag="ps")
                    mm(pe[:], wtile[:kdim, mc * 128:(mc + 1) * 128], xtile[:kdim, :],
                       start=True, stop=False)
                    for it in range(BL):
                        nc.tensor.matmul(
                            pe[:, it * T:(it + 1) * T],
                            te[it][:, mc * 128:(mc + 1) * 128], ident[:],
                            is_transpose=True, start=False, stop=(it == BL - 1))
                    # h[:, mc, off::3] covers (item,t) in order
                    nc.vector.tensor_scalar(
                        out=h[:, mc, off::3], in0=pe[:],
                        scalar1=eb[:, off, mc:mc + 1], scalar2=None, op0=AL.add)

            # ================= layernorm helper =================
            def layernorm(x, sc_ap_fn, bi_ap_fn):
                """In-place LN over partition-H on x [128, KC, W2]."""
                xsq = p_hid.tile([128, 4, W2], MMDT, tag="hid")
                for kc in range(KC):
                    nc.scalar.activation(xsq[:, kc, :], x[:, kc, :], AF.Square)
                for it in range(BL):
                    sl = slice(it * S, (it + 1) * S)
                    s1 = p_ps.tile([1, S], F32, tag="ps")
                    s2 = p_ps.tile([1, S], F32, tag="ps")
                    for kc in range(KC):
                        mm(s1[:], ones_col[:, :1], x[:, kc, sl], start=(kc == 0), stop=(kc == KC - 1))
                    for kc in range(KC):
                        mm(s2[:], ones_col[:, :1], xsq[:, kc, sl], start=(kc == 0), stop=(kc == KC - 1))
                    # broadcast stats to 128 partitions, then do all math wide
                    s1c = p_sm.tile([1, S], MMDT, tag="s1c")
                    nc.scalar.copy(s1c[:, :], s1[:])
                    s2c = p_sm.tile([1, S], MMDT, tag="s2c")
                    nc.scalar.copy(s2c[:, :], s2[:])
                    mb = p_ps.tile([128, S], F32, tag="ps")
                    mm(mb[:], ones_h[:1, :], s1c[:, :], start=True, stop=True)   # mean bcast
                    sb2 = p_ps.tile([128, S], F32, tag="ps")
                    mm(sb2[:], ones_row[:1, :], s2c[:, :], start=True, stop=True)  # sumsq bcast
                    msqb = p_sm.tile([128, S], F32, tag="msqb")
                    nc.scalar.activation(msqb[:], mb[:], AF.Square)
                    ub = p_sm.tile([128, S], F32, tag="ub")
                    nc.vector.scalar_tensor_tensor(out=ub[:], in0=sb2[:], scalar=1.0 / H,
                                                   in1=msqb[:], op0=AL.mult, op1=AL.subtract)
                    sdb = p_sm.tile([128, S], F32, tag="sdb")
                    nc.scalar.activation(sdb[:], ub[:], AF.Ln, bias=eps128[:, :1])
                    rstd = p_sm.tile([128, S], F32, tag="rstd")
                    nc.scalar.activation(rstd[:], sdb[:], AF.Exp, scale=-0.5)
                    for kc in range(KC):
                        nc.vector.tensor_tensor(out=x[:, kc, sl], in0=x[:, kc, sl],
                                                in1=mb[:], op=AL.subtract)
                        nc.vector.tensor_tensor(out=x[:, kc, sl], in0=x[:, kc, sl],
                                                in1=rstd[:], op=AL.mult)
                        nc.vector.tensor_scalar(out=x[:, kc, sl], in0=x[:, kc, sl],
                                                scalar1=sc_ap_fn(kc), scalar2=bi_ap_fn(kc),
                                                op0=AL.mult, op1=AL.add)

            layernorm(h, lambda kc: embln_sb[:, 0, kc:kc + 1], lambda kc: embln_sb[:, 1, kc:kc + 1])

            # ================= transformer blocks =================
            for bi in range(n_blocks):
                bw = blk_w[bi]
                bb = bqkvo[bi]
                # --- q/k projections (transposed layout) ---
                qt = p_qk.tile([128, KC, W2], MMDT, tag="qk")
                kt = p_qk.tile([128, KC, W2], MMDT, tag="qk")
                for pi, dst in ((0, qt), (1, kt)):
                    wch = p_wgt.tile([128, KC, H], MMDT, tag="wgt")
                    nc.sync.dma_start(wch[:], bw['qkv'][:, pi].bitcast(MMDT))
                    for mc in range(KC):
                        for it in range(BL):
                            pp = p_ps.tile([128, S], F32, tag="ps")
                            for kc in range(KC):
                                mm(pp[:], wch[:, kc, mc * 128:(mc + 1) * 128],
                                   h[:, kc, it * S:(it + 1) * S],
                                   start=(kc == 0), stop=(kc == KC - 1))
                            nc.vector.tensor_scalar(
                                out=dst[:, mc, it * S:(it + 1) * S], in0=pp[:],
                                scalar1=bb[:, pi, mc:mc + 1], scalar2=None, op0=AL.add)
                # --- v in natural layout [tok, H] ---
                vch = p_wgt.tile([128, KC, H], MMDT, tag="wgt")
                nc.sync.dma_start(vch[:], bw['qkv'][:, 2].bitcast(MMDT))
                vn = p_vn.tile([128, BL * 3, H], MMDT, tag="vn")
                for it in range(BL):
                    for tt in range(3):
                        pp = p_ps.tile([128, H], F32, tag="ps")
                        for kc in range(KC):
                            mm(pp[:], h[:, kc, it * S + tt * 128: it * S + (tt + 1) * 128],
                               vch[:, kc, :], start=(kc == 0), stop=(kc == KC - 1))
                        nc.scalar.copy(vn[:, it * 3 + tt, :], pp[:])
                # --- attention per item/head ---
                aot = p_qk.tile([128, KC, W2], MMDT, tag="qk")
                for it in range(BL):
                    for hp in range(NH // 2):
                        ao_ps = []
                        for sub in range(2):
                            hd = hp * 2 + sub
                            kc_h = hd // 2
                            pb = 64 * (hd & 1)
                            q_ap = qt[pb:pb + 64, kc_h, it * S:(it + 1) * S]
                            k_ap = kt[pb:pb + 64, kc_h, it * S:(it + 1) * S]
                            wsb = p_w.tile([128, 3, S], dt.float16, tag="wsb")
                            nc.gpsimd.memset(wsb[:, 0, 128:S], 0.0)
                            nc.gpsimd.memset(wsb[:, 1, 256:S], 0.0)
                            rs = p_sm.tile([128, 3], F32, tag="rs")
                            for tt in range(3):
                                span = (tt + 1) * 128
                                sc = p_ps.tile([128, S], F32, tag="ps")
                                mm(sc[:, :span], q_ap[:, tt * 128:(tt + 1) * 128],
                                   k_ap[:, :span], start=True, stop=True)
                                nc.vector.tensor_tensor(
                                    out=sc[:, tt * 128:span], in0=sc[:, tt * 128:span],
                                    in1=cmask[:], op=AL.add)
                                nc.scalar.activation(wsb[:, tt, :span], sc[:, :span],
                                                     AF.Exp, accum_out=rs[:, tt:tt + 1])
                            rr = p_sm.tile([128, 3], F32, tag="rr")
                            nc.vector.reciprocal(rr[:], rs[:])
                            for tt in range(3):
                                span = (tt + 1) * 128
                                nc.vector.tensor_scalar(
                                    out=wsb[:, tt, :span], in0=wsb[:, tt, :span],
                                    scalar1=rr[:, tt:tt + 1], scalar2=None, op0=AL.mult)
                            nc.sync.dma_start(
                                attns[bi, it, hd].rearrange("(c p) f -> p c f", p=128),
                                wsb[:])
                            # transpose w -> wT tiles
                            wt = p_wt.tile([128, 3, S], MMDT, tag="wtsb")
                            for ft in range(3):
                                tspan = S - ft * 128
                                tp = p_ps.tile([128, S], dt.float16, tag="ps")
                                for tt in range(ft, 3):
                                    nc.tensor.matmul(
                                        tp[:, (tt - ft) * 128:(tt - ft + 1) * 128],
                                        wsb[:, tt, ft * 128:(ft + 1) * 128],
                                        ident16[:], is_transpose=True,
                                        start=(tt == ft), stop=(tt == 2))
                                nc.vector.tensor_copy(wt[:, ft, :tspan], tp[:, :tspan])
                            # attn @ v -> [64, S] in psum (col-packed pairs)
                            ap_ = p_ps.tile([128, S], F32, tag="ps")
                            ao_ps.append((ap_, wt))
                            for ft in range(3):
                                tspan = S - ft * 128
                                mm(ap_[pb:pb + 64, ft * 128:S],
                                   vn[:, it * 3 + ft, hd * 64:(hd + 1) * 64],
                                   wt[:, ft, :tspan],
                                   start=(ft == 0), stop=(ft == 2),
                                   tile_position=(0, pb))
                        # evacuate pair into aot with v-bias fold
                        for sub in range(2):
                            hd = hp * 2 + sub
                            ap_, _ = ao_ps[sub]
                            pb = 64 * (hd & 1)
                            nc.vector.tensor_scalar(
                                out=aot[pb:pb + 64, hd // 2, it * S:(it + 1) * S],
                                in0=ap_[pb:pb + 64, :],
                                scalar1=bb[pb:pb + 64, 2, (hd // 2):(hd // 2) + 1],
                                scalar2=None, op0=AL.add)
                # --- output projection + residual ---
                och = p_wgt.tile([128, KC, H], MMDT, tag="wgt")
                nc.sync.dma_start(och[:], bw['o'].bitcast(MMDT))
                h2 = p_act.tile([128, KC, W2], MMDT, tag="act")
                for mc in range(KC):
                    for it in range(BL):
                        pp = p_ps.tile([128, S], F32, tag="ps")
                        for kc in range(KC):
                            mm(pp[:], och[:, kc, mc * 128:(mc + 1) * 128],
                               aot[:, kc, it * S:(it + 1) * S],
                               start=(kc == 0), stop=(kc == KC - 1))
                        nc.vector.scalar_tensor_tensor(
                            out=h2[:, mc, it * S:(it + 1) * S], in0=pp[:],
                            scalar=bb[:, 3, mc:mc + 1],
                            in1=h[:, mc, it * S:(it + 1) * S],
                            op0=AL.add, op1=AL.add)
                h = h2
                ln = lns[bi]
                layernorm(h, lambda kc: ln[:, 0, 0, kc:kc + 1], lambda kc: ln[:, 0, 1, kc:kc + 1])
                # --- ffn ---
                h3 = p_act.tile([128, KC, W2], MMDT, tag="act")
                b1 = bfc1[bi]
                for hc in range(4):
                    f1 = p_wgt.tile([128, KC, H], MMDT, tag="wgt")
                    nc.sync.dma_start(f1[:], bw['fc1'][:, :, hc * 512:(hc + 1) * 512].bitcast(MMDT))
                    hid = p_hid.tile([128, 4, W2], MMDT, tag="hid")
                    for hm in range(4):
                        for it in range(BL):
                            pp = p_ps.tile([128, S], F32, tag="ps")
                            for kc in range(KC):
                                mm(pp[:], f1[:, kc, hm * 128:(hm + 1) * 128],
                                   h[:, kc, it * S:(it + 1) * S],
                                   start=(kc == 0), stop=(kc == KC - 1))
                            nc.scalar.activation(
                                hid[:, hm, it * S:(it + 1) * S], pp[:],
                                AF.Gelu_apprx_tanh,
                                bias=b1[:, hc * 4 + hm:hc * 4 + hm + 1])
                    f2 = p_wgt.tile([128, 4, H], MMDT, tag="wgt")
                    nc.sync.dma_start(f2[:], bw['fc2'][:, hc * 4:(hc + 1) * 4, :].bitcast(MMDT))
                    for mc in range(KC):
                        for it in range(BL):
                            pp2 = p_ps.tile([128, S], F32, tag="ps")
                            for kk in range(4):
                                mm(pp2[:], f2[:, kk, mc * 128:(mc + 1) * 128],
                                   hid[:, kk, it * S:(it + 1) * S],
                                   start=(kk == 0), stop=(kk == 3))
                            if hc == 0:
                                # h3 = x1 + psum + b2 (first chunk: include residual+bias)
                                nc.vector.scalar_tensor_tensor(
                                    out=h3[:, mc, it * S:(it + 1) * S], in0=pp2[:],
                                    scalar=bfc2[bi][:, mc:mc + 1],
                                    in1=h[:, mc, it * S:(it + 1) * S],
                                    op0=AL.add, op1=AL.add)
                            else:
                                nc.vector.tensor_tensor(
                                    out=h3[:, mc, it * S:(it + 1) * S],
                                    in0=h3[:, mc, it * S:(it + 1) * S],
                                    in1=pp2[:], op=AL.add)
                h = h3
                layernorm(h, lambda kc: ln[:, 1, 0, kc:kc + 1], lambda kc: ln[:, 1, 1, kc:kc + 1])

            if debug_h:
                nc.sync.dma_start(hdbg, h[:])

            # ================= tail: heads + align attention =================
            if tail:
                def rep_ap(kc, off, it=None):
                    """strided stream columns; it=None -> both items [128, 2, T]"""
                    if it is None:
                        return h[:, kc, :].rearrange("p (i t) -> p i t", i=BL)[:, :, off::3]
                    return h[:, kc, it * S + off:(it + 1) * S:3]

                # prediction heads from action_repr (off=2) and state_repr (off=1)
                for wt_, bt_, od, outdim, off, act in (
                        (ps_w, ps_b, sp_o, SD, 2, None),
                        (pr_w, pr_b, rp_o, 1, 2, None),
                        (pa_w, pa_b, ap_o, AD, 1, AF.Tanh)):
                    wsb_ = p_cn.tile([128, KC, outdim], MMDT, tag=f"hw{outdim}_{off}")
                    nc.sync.dma_start(wsb_[:], wt_.bitcast(MMDT))
                    bsb_ = p_cn.tile([outdim, 1], F32, tag=f"hb{outdim}_{off}")
                    nc.sync.dma_start(bsb_[:], bt_)
                    for it in range(BL):
                        pp = p_ps.tile([outdim, T], F32, tag="ps")
                        for kc in range(KC):
                            mm(pp[:], wsb_[:, kc, :], rep_ap(kc, off, it),
                               start=(kc == 0), stop=(kc == KC - 1))
                        ot = p_sm.tile([outdim, T], F32, tag=f"ho{outdim}_{off}")
                        if act is None:
                            nc.vector.tensor_scalar(out=ot[:], in0=pp[:],
                                                    scalar1=bsb_[:, :1], scalar2=None, op0=AL.add)
                        else:
                            nc.scalar.activation(ot[:], pp[:], act, bias=bsb_[:, :1])
                        nc.sync.dma_start(od[it], ot[:])

                # --- align attention (q from state/action reprs, kv from return repr) ---
                ab = p_cn.tile([128, 4, 4], F32, tag="alb")
                nc.sync.dma_start(ab[:], al_qkvo_b)
                # k,v,qs,qa transposed [128, KC, 2, T]
                kt_a = p_qk.tile([128, KC, W2], MMDT, tag="qk")
                q_s = p_qk.tile([128, KC, W2], MMDT, tag="qk")
                q_a = p_qk.tile([128, KC, W2], MMDT, tag="qk")

                def proj_t(dst, wch, pi, off):
                    for mc in range(KC):
                        pp = p_ps.tile([128, BL * T], F32, tag="ps")
                        for kc in range(KC):
                            mm(pp[:], wch[:, kc, mc * 128:(mc + 1) * 128], rep_ap(kc, off),
                               start=(kc == 0), stop=(kc == KC - 1))
                        nc.vector.tensor_scalar(
                            out=dst[:, mc, :BL * T], in0=pp[:],
                            scalar1=ab[:, pi, mc:mc + 1], scalar2=None, op0=AL.add)

                aw = p_wgt.tile([128, KC, H], MMDT, tag="wgt")
                nc.sync.dma_start(aw[:], al_qkv_w[:, 0].bitcast(MMDT))
                proj_t(q_s, aw, 0, 1)
                proj_t(q_a, aw, 0, 2)
                aw2 = p_wgt.tile([128, KC, H], MMDT, tag="wgt")
                nc.sync.dma_start(aw2[:], al_qkv_w[:, 1].bitcast(MMDT))
                proj_t(kt_a, aw2, 1, 0)
                # v natural per item [T, H]
                aw3 = p_wgt.tile([128, KC, H], MMDT, tag="wgt")
                nc.sync.dma_start(aw3[:], al_qkv_w[:, 2].bitcast(MMDT))
                vn_a = p_vn.tile([128, BL * 3, H], MMDT, tag="vn")
                for it in range(BL):
                    pp = p_ps.tile([128, H], F32, tag="ps")
                    for kc in range(KC):
                        mm(pp[:], rep_ap(kc, 0, it), aw3[:, kc, :],
                           start=(kc == 0), stop=(kc == KC - 1))
                    nc.scalar.copy(vn_a[:, it * 3, :], pp[:])

                ow = p_wgt.tile([128, KC, H], MMDT, tag="wgt")
                nc.sync.dma_start(ow[:], al_o_w.bitcast(MMDT))
                pw = p_wgt.tile([128, KC, H], MMDT, tag="wgt")
                nc.sync.dma_start(pw[:], al_p_w.bitcast(MMDT))
                apb = p_cn.tile([128, 4], F32, tag="apb")
                nc.sync.dma_start(apb[:], al_p_b)

                for qsrc, od in ((q_s, sproj_o), (q_a, aproj_o)):
                    cnd = p_cnd.tile([128, KC, W2], F32, tag="cnd")
                    for it in range(BL):
                        # attention: heads
                        aot2 = p_w.tile([128, KC, T], MMDT, tag="alao")
                        for hd in range(NH):
                            kc_h = hd // 2
                            pb = 64 * (hd & 1)
                            q_ap = qsrc[pb:pb + 64, kc_h, it * T:(it + 1) * T]
                            k_ap = kt_a[pb:pb + 64, kc_h, it * T:(it + 1) * T]
                            sc = p_ps.tile([128, T], F32, tag="ps")
                            mm(sc[:], q_ap, k_ap, start=True, stop=True)
                            nc.vector.tensor_tensor(out=sc[:], in0=sc[:], in1=cmask[:], op=AL.add)
                            u = p_sm.tile([128, T], dt.float16, tag="alu")
                            rs = p_sm.tile([128, 1], F32, tag="alrs")
                            nc.scalar.activation(u[:], sc[:], AF.Exp, accum_out=rs[:, :1])
                            rr = p_sm.tile([128, 1], F32, tag="alrr")
                            nc.vector.reciprocal(rr[:], rs[:])
                            nc.vector.tensor_scalar(out=u[:], in0=u[:], scalar1=rr[:, :1],
                                                    scalar2=None, op0=AL.mult)
                            tp = p_ps.tile([128, T], dt.float16, tag="ps")
                            nc.tensor.matmul(tp[:], u[:], ident16[:],
                                             is_transpose=True, start=True, stop=True)
                            ut = p_sm.tile([128, T], MMDT, tag="alut")
                            nc.vector.tensor_copy(ut[:], tp[:])
                            av = p_ps.tile([128, T], F32, tag="ps")
                            mm(av[pb:pb + 64, :], vn_a[:, it * 3, hd * 64:(hd + 1) * 64],
                               ut[:], start=True, stop=True, tile_position=(0, pb))
                            nc.vector.tensor_scalar(
                                out=aot2[pb:pb + 64, kc_h, :], in0=av[pb:pb + 64, :],
                                scalar1=ab[pb:pb + 64, 2, kc_h:kc_h + 1],
                                scalar2=None, op0=AL.add)
                        # o proj -> cond, then align_proj -> cnd
                        condt = p_wt.tile([128, KC, T], MMDT, tag="alcond")
                        for mc in range(KC):
                            pp = p_ps.tile([128, T], F32, tag="ps")
                            for kc in range(KC):
                                mm(pp[:], ow[:, kc, mc * 128:(mc + 1) * 128], aot2[:, kc, :],
                                   start=(kc == 0), stop=(kc == KC - 1))
                            nc.vector.tensor_scalar(
                                out=condt[:, mc, :], in0=pp[:],
                                scalar1=ab[:, 3, mc:mc + 1], scalar2=None, op0=AL.add)
                        for mc in range(KC):
                            pp = p_ps.tile([128, T], F32, tag="ps")
                            for kc in range(KC):
                                mm(pp[:], pw[:, kc, mc * 128:(mc + 1) * 128], condt[:, kc, :],
                                   start=(kc == 0), stop=(kc == KC - 1))
                            nc.vector.tensor_scalar(
                                out=cnd[:, mc, it * T:it * T + T], in0=pp[:],
                                scalar1=apb[:, mc:mc + 1], scalar2=None, op0=AL.add)
                    # l2 normalize over H and write out
                    csq = p_hid.tile([128, KC, W2], MMDT, tag="hid")
                    for kc in range(KC):
                        nc.scalar.activation(csq[:, kc, :BL * T], cnd[:, kc, :BL * T], AF.Square)
                    s2 = p_ps.tile([1, BL * T], F32, tag="ps")
                    for kc in range(KC):
                        mm(s2[:], ones_col[:, :1], csq[:, kc, :BL * T],
                           start=(kc == 0), stop=(kc == KC - 1))
                    nrm = p_sm.tile([1, BL * T], F32, tag="nrm")
                    nc.scalar.activation(nrm[:], s2[:], AF.Sqrt)
                    lnn = p_sm.tile([1, BL * T], F32, tag="lnn")
                    nc.scalar.activation(lnn[:], nrm[:], AF.Ln, bias=eps8_t[:1, :1])
                    rn = p_sm.tile([1, BL * T], MMDT, tag="rn")
                    nc.scalar.activation(rn[:], lnn[:], AF.Exp, scale=-1.0)
                    nb_ = p_ps.tile([128, BL * T], F32, tag="ps")
                    mm(nb_[:], ones_row[:1, :], rn[:1, :], start=True, stop=True)
                    for kc in range(KC):
                        nc.vector.tensor_tensor(out=cnd[:, kc, :BL * T], in0=cnd[:, kc, :BL * T],
                                                in1=nb_[:], op=AL.mult)
                    for it in range(BL):
                        nc.sync.dma_start(
                            od[it].rearrange("(c p) t -> p c t", p=128),
                            cnd[:, :, it * T:(it + 1) * T])

    nc.compile()
    return nc


def _get(key, **kw):
    if key not in _BUILT:
        _BUILT[key] = _build(**kw)
    return _BUILT[key]


# ----------------------------------------------------------------------------
# entry point
# ----------------------------------------------------------------------------

def kernel(timesteps, states, actions, returns_to_go, params, _trace=False, _tmpdir=None):
    from concourse.bass_utils import run_bass_kernel_spmd

    nc = _get('full')
    in_maps = _pack_inputs(timesteps, states, actions, returns_to_go, params)
    res = run_bass_kernel_spmd(nc, in_maps, list(range(NC)), trace=_trace, tmpdir=_tmpdir)
    kernel._last = res

    outs = res.results
    attns = np.concatenate([o['attns'] for o in outs], axis=1).astype(np.float32)
    sp = np.concatenate([o['sp'] for o in outs], 0).transpose(0, 2, 1)  # [16,128,17]
    ap_ = np.concatenate([o['ap_'] for o in outs], 0).transpose(0, 2, 1)
    rp = np.concatenate([o['rp'] for o in outs], 0).transpose(0, 2, 1)
    sproj = np.concatenate([o['sproj'] for o in outs], 0).transpose(0, 2, 1)
    aproj = np.concatenate([o['aproj'] for o in outs], 0).transpose(0, 2, 1)
    return sp, ap_, rp, attns, (sproj, aproj)
